# revision 19
# baseline (speedup 1.0000x reference)
"""MinGRU cell on 8 Trainium2 NeuronCores (Bass/Tile).

Math (per batch b, hidden h):
    gz = x @ W_z^T ; gh = x @ W_h^T                 (two GEMMs, K=D=1024)
    z  = sigmoid(gz + b_z)
    h_t = (1 - z_t) * h_{t-1} + z_t * (gh_t + b_h)  (affine scan over T)

Distribution: data-parallel over batch B=16 -> 2 batches per core, weights
replicated; no cross-core communication.

Per-core steady state: the PE streams the GEMMs (128 N=512 bf16 matmuls
per 512-token step, ~218 ns each = the bf16 roofline) with the previous
step's 32 cheap out-transposes (56 ns) interleaved between GEMM groups.
x^T tiles for steps 1..7 are produced by the DMA crossbar
(dma_start_transpose) straight from (host-precast bf16) DRAM, issued a
full step ahead — the crossbar's completion semaphore has been observed
to lead its data on profiled runs, so every crossbar transpose here has
>15 us between issue and first consumer. Step 0's x^T is built on the PE
instead (plain loads + tensor-engine transposes), which both avoids that
settle window and lets the first GEMM start as early as the weights land.
W arrives pre-transposed bf16 from the host (weight pre-packing) in four
half-H strided DMAs that lead the two HWDGE queues; bias/h0 gathers go to
the GpSimd SWDGE queue. ACT runs the two sigmoids (z and 1-z), DVE the
(gh+b_h)*z fuse and the affine scan. Output is written bf16 and upcast to
f32 on the host (it was computed in bf16 either way).
"""

import sys

sys.path.insert(0, "/opt/trn_rl_repo")

from contextlib import ExitStack

import numpy as np
import ml_dtypes

import concourse.bass as bass
import concourse.mybir as mybir
import concourse.tile as tile
from concourse import bacc
from concourse.bass import ts, ds
from concourse.bass_utils import run_bass_kernel_spmd
from concourse.masks import make_identity

B, T, D, H = 16, 2048, 1024, 1024
NCORES = 8
B_LOC = B // NCORES  # 2
P = 128
TC = 512  # tokens per step
NSTEP = B_LOC * T // TC  # 8
NTC = T // TC  # 4 steps per batch
TSUB = TC // P  # 4
DC = D // P  # 8 contraction chunks
HC = H // P  # 8 hidden chunks
HQ = H // 4  # 256, one h-quarter of W per DMA

F32 = mybir.dt.float32
BF16 = mybir.dt.bfloat16
AF = mybir.ActivationFunctionType
OP = mybir.AluOpType

_CACHE = {}


def _mingru_tile(tc, out, x, h0, wzT, bz, whT, bh):
    nc = tc.nc

    with ExitStack() as ctx:
        consts = ctx.enter_context(tc.tile_pool(name="consts", bufs=1))

        id_bf = consts.tile([P, P], BF16)
        make_identity(nc, id_bf)

        # Small strided gathers on the otherwise idle SWDGE queue.
        bz_sb = consts.tile([P, HC], F32)
        nc.gpsimd.dma_start(out=bz_sb, in_=bz.rearrange("(c p) -> p c", p=P))
        bh_sb = consts.tile([P, HC], F32)
        nc.gpsimd.dma_start(out=bh_sb, in_=bh.rearrange("(c p) -> p c", p=P))
        hp_sb = consts.tile([P, B_LOC * HC], F32)
        nc.gpsimd.dma_start(out=hp_sb, in_=h0.rearrange("b (c p) -> p (b c)", p=P))
        nbz_sb = consts.tile([P, HC], F32)
        nc.vector.tensor_scalar_mul(nbz_sb, bz_sb, -1.0)

        xt_p = ctx.enter_context(tc.tile_pool(name="xt", bufs=2))
        azb_p = ctx.enter_context(tc.tile_pool(name="azb", bufs=2))
        scan_p = ctx.enter_context(tc.tile_pool(name="scan", bufs=2))
        onat_p = ctx.enter_context(tc.tile_pool(name="onat", bufs=2))
        xnat_p = ctx.enter_context(tc.tile_pool(name="xnat", bufs=1))

        def step_bt(s):
            return s // NTC, s % NTC

        # W^T arrives pre-transposed [D, H] bf16 from the host. One strided
        # DMA per h-quarter; the first quarters lead each HWDGE queue (W_z
        # on SP, W_h on ACT) so the first GEMM only waits on 0.5 MB of W:
        #   wt[wn][r][p, dc*HQ + h'] = W^T[dc*128 + p, r*HQ + h']
        # lhsT block (hc,dc) = wt[wn][hc//2][:, dc*HQ + (hc%2)*128 ...].
        wt = {"z": [], "h": []}

        def w_load(wn, r):
            w_ap, w_eng = (wzT, nc.sync) if wn == "z" else (whT, nc.scalar)
            w_sb = consts.tile([P, DC * HQ], BF16, name=f"wt_{wn}{r}")
            w_eng.dma_start(
                out=w_sb.rearrange("p (dc h) -> p dc h", h=HQ),
                in_=w_ap[:, ds(r * HQ, HQ)].rearrange("(dc p) h -> p dc h", p=P),
            )
            wt[wn].append(w_sb)

        # Step 0's x leads both HWDGE queues (the first GEMM needs all of
        # it, but only the first W quarter), then the W quarters stream in
        # first-used-first.
        xn0 = []
        for j in range(TSUB):
            t_ = xnat_p.tile([P, D], BF16, tag=f"xn{j}", name=f"xn0_{j}")
            eng = nc.scalar if j % 2 else nc.sync
            eng.dma_start(out=t_, in_=x[0, ds(j * P, P), :])
            xn0.append(t_)

        for r in range(4):
            w_load("z", r)
            w_load("h", r)

        xts = {}

        def t_x(s):  # crossbar transpose, issued a full step ahead of use
            b, tci = step_bt(s)
            tiles = []
            for dc in range(DC):
                t_ = xt_p.tile([P, TC], BF16, tag=f"xt{dc}", name=f"xt_{s}_{dc}")
                nc.sync.dma_start_transpose(t_, x[b, ds(tci * TC, TC), ts(dc, P)])
                tiles.append(t_)
            xts[s] = tiles

        t_x(1)

        # Prologue PE work: HAM warmup junk, then step 0's x^T on the PE
        # (transpose to PSUM, ACT copy to SBUF).
        xts[0] = []
        with tc.tile_pool(name="warm", bufs=1, space="PSUM") as warm_p, \
             tc.tile_pool(name="pxt", bufs=2, space="PSUM") as pxt_p, \
             tc.tile_pool(name="wdram", bufs=1, space="DRAM") as wdram_p:
            junk_ps = warm_p.tile([P, P], F32, name="junk_ps")
            NWARM = 30
            for i in range(NWARM):
                nc.tensor.matmul(
                    junk_ps, id_bf, id_bf, start=(i == 0), stop=(i == NWARM - 1)
                )
            junk_sb = consts.tile([P, P], F32, name="junk_sb")
            nc.vector.tensor_copy(junk_sb, junk_ps)
            junk_dr = wdram_p.tile([P, P], F32, name="junk_dr")
            nc.sync.dma_start(out=junk_dr, in_=junk_sb)

            for dc in range(DC):
                pxt = pxt_p.tile([P, TC], BF16, tag="pxt", name=f"pxt_{dc}")
                for j in range(TSUB):
                    nc.tensor.transpose(pxt[:, ts(j, P)], xn0[j][:, ts(dc, P)], id_bf)
                xt_sb = xt_p.tile([P, TC], BF16, tag=f"xt{dc}", name=f"xt_0_{dc}")
                nc.scalar.copy(xt_sb, pxt)
                xts[0].append(xt_sb)

        # PSUM: 3 z + 3 h GEMM banks, 2 for the PE out-transposes.
        pz_p = ctx.enter_context(tc.tile_pool(name="pz", bufs=3, space="PSUM"))
        ph_p = ctx.enter_context(tc.tile_pool(name="ph", bufs=3, space="PSUM"))
        po_p = ctx.enter_context(tc.tile_pool(name="po", bufs=2, space="PSUM"))

        scans = {}

        def gemm(s, hc, wn):
            pool = pz_p if wn == "z" else ph_p
            psum = pool.tile([P, TC], F32, tag="p" + wn, name=f"ps{wn}_{s}_{hc}")
            xt = xts[s]
            w_sb = wt[wn][hc // 2]
            for dc in range(DC):
                nc.tensor.matmul(
                    psum,
                    w_sb[:, ds(dc * HQ + (hc % 2) * P, P)],
                    xt[dc],
                    start=(dc == 0),
                    stop=(dc == DC - 1),
                )
            return psum

        def post(s, hc, psum_z, psum_h):
            b, tci = step_bt(s)
            a_sb = azb_p.tile([P, TC], F32, tag="a", name=f"a_{s}_{hc}")
            nc.scalar.activation(
                a_sb, psum_z, AF.Sigmoid, bias=nbz_sb[:, hc : hc + 1], scale=-1.0
            )
            z_sb = azb_p.tile([P, TC], F32, tag="z", name=f"z_{s}_{hc}")
            nc.scalar.activation(
                z_sb, psum_z, AF.Sigmoid, bias=bz_sb[:, hc : hc + 1], scale=1.0
            )
            bsc = azb_p.tile([P, TC], F32, tag="b", name=f"b_{s}_{hc}")
            nc.vector.scalar_tensor_tensor(
                bsc, psum_h, bh_sb[:, hc : hc + 1], z_sb, op0=OP.add, op1=OP.mult
            )
            # bf16 scan output: the scan accumulator is fp32 in HW
            # regardless of out dtype, so only stored values round; bf16
            # keeps the PE out-transposes at 1 cyc/row.
            sc = scan_p.tile([P, TC], BF16, tag=f"sc{hc}", name=f"sc_{s}_{hc}")
            if tci == 0:
                init = hp_sb[:, b * HC + hc : b * HC + hc + 1]
            else:
                init = scans[s - 1][hc][:, TC - 1 : TC]
            nc.vector.tensor_tensor_scan(sc, a_sb, bsc, init, op0=OP.mult, op1=OP.add)
            scans.setdefault(s, [None] * HC)[hc] = sc

        # Out path, emitted in per-(j,half) blocks so it can interleave
        # between GEMM groups: 4 PE transposes -> po PSUM -> copy to the
        # bf16 staging tile -> one store per j.
        on_tiles = {}

        def out_block(s, j, half):
            if half == 0:
                on_tiles[s, j] = onat_p.tile(
                    [P, H], BF16, tag=f"on{j}", name=f"on_{s}_{j}"
                )
            on = on_tiles[s, j]
            scs = scans[s]
            po = po_p.tile([P, TC], BF16, tag="po", name=f"po_{s}_{j}_{half}")
            for k in range(4):
                hc = half * 4 + k
                nc.tensor.transpose(po[:, ts(k, P)], scs[hc][:, ts(j, P)], id_bf)
            if half == 0:
                nc.scalar.copy(on[:, ds(0, TC)], po)
            else:
                nc.vector.tensor_copy(on[:, ds(TC, TC)], po)

        def out_store(s, j):
            b, tci = step_bt(s)
            nc.sync.dma_start(
                out=out[b, ds(tci * TC + j * P, P), :], in_=on_tiles.pop((s, j))
            )

        # --- steps -------------------------------------------------------
        for s in range(NSTEP):
            if s == 0:
                # W_h^T lands after W_z^T: all z GEMMs first.
                pzs = [gemm(0, hc, "z") for hc in range(HC)]
                phs = [gemm(0, hc, "h") for hc in range(HC)]
                for hc in range(HC):
                    post(0, hc, pzs[hc], phs[hc])
            else:
                if s + 1 < NSTEP:
                    t_x(s + 1)
                for hc in range(HC):
                    psum_z = gemm(s, hc, "z")
                    psum_h = gemm(s, hc, "h")
                    post(s, hc, psum_z, psum_h)
                    # previous step's out path, spread across this step
                    out_block(s - 1, hc // 2, hc % 2)
                    if hc % 2 == 1:
                        out_store(s - 1, hc // 2)
                if s - 2 in scans:
                    del scans[s - 2]
        # Tail: all half-0 blocks first — they only need scans 0..3 and run
        # on the PE while the last scans drain on DVE; then half-1 + stores.
        s = NSTEP - 1
        for j in range(TSUB):
            out_block(s, j, 0)
        for j in range(TSUB):
            out_block(s, j, 1)
            out_store(s, j)


def build():
    if "nc" in _CACHE:
        return _CACHE["nc"]
    nc = bacc.Bacc(
        "TRN2", target_bir_lowering=False, debug=False, num_devices=NCORES
    )
    x = nc.dram_tensor("x", [B_LOC, T, D], BF16, kind="ExternalInput").ap()
    h0 = nc.dram_tensor("h0", [B_LOC, H], F32, kind="ExternalInput").ap()
    wzT = nc.dram_tensor("wzT", [D, H], BF16, kind="ExternalInput").ap()
    bz = nc.dram_tensor("bz", [H], F32, kind="ExternalInput").ap()
    whT = nc.dram_tensor("whT", [D, H], BF16, kind="ExternalInput").ap()
    bh = nc.dram_tensor("bh", [H], F32, kind="ExternalInput").ap()
    out = nc.dram_tensor("out", [B_LOC, T, H], BF16, kind="ExternalOutput").ap()
    with tile.TileContext(nc) as tctx:
        _mingru_tile(tctx, out, x, h0, wzT, bz, whT, bh)
    nc.compile()
    _CACHE["nc"] = nc
    return nc


def make_in_maps(x, h_prev, W_z, b_z, W_h, b_h):
    x = np.asarray(x, dtype=np.float32).astype(ml_dtypes.bfloat16)
    h_prev = np.ascontiguousarray(np.asarray(h_prev, dtype=np.float32))
    wzT = np.asarray(W_z, dtype=np.float32).T.astype(ml_dtypes.bfloat16)
    whT = np.asarray(W_h, dtype=np.float32).T.astype(ml_dtypes.bfloat16)
    b_z = np.ascontiguousarray(np.asarray(b_z, dtype=np.float32))
    b_h = np.ascontiguousarray(np.asarray(b_h, dtype=np.float32))
    in_maps = []
    for c in range(NCORES):
        sl = slice(c * B_LOC, (c + 1) * B_LOC)
        in_maps.append(
            {
                "x": np.ascontiguousarray(x[sl]),
                "h0": h_prev[sl],
                "wzT": wzT,
                "bz": b_z,
                "whT": whT,
                "bh": b_h,
            }
        )
    return in_maps


def kernel(x, h_prev, W_z, b_z, W_h, b_h, trace=False):
    nc = build()
    in_maps = make_in_maps(x, h_prev, W_z, b_z, W_h, b_h)
    res = run_bass_kernel_spmd(
        nc, in_maps, core_ids=list(range(NCORES)), trace=trace
    )
    out = np.concatenate(
        [np.asarray(r["out"]).astype(np.float32) for r in res.results], axis=0
    )
    if trace:
        _CACHE["last_results"] = res
    return out
